# revision 24
# baseline (speedup 1.0000x reference)
"""CfC cell (dense MLP) Trainium2 Bass kernel.

Reference math (fp32):
    x  = concat([input, hx], axis=1)                  # [B, 768]
    h  = 1.7159 * tanh(0.666 * (x @ Wb.T + bb))       # [B, 1024]
    ff1 = tanh(h @ W1.T + b1)                         # [B, 512]
    ff2 = tanh(h @ W2.T + b2)
    t_a = h @ Wa.T + ba
    t_b = h @ Wt.T + bt
    t   = sigmoid(t_a * ts + t_b)
    out = ff1 * (1 - t) + t * ff2

Strategy: data-parallel over batch across 8 NeuronCores (2048 rows each).
Layer 1 (backbone) and the ff1/ff2 heads run in fp16 (accuracy-critical:
their error passes straight to the output through tanh). The t-path heads
(Wa, Wt) run in fp8-e4m3 with MatmulPerfMode.DoubleRow - 2x PE throughput -
because their quantization error is damped by the sigmoid slope (<=0.25)
and the (ff2-ff1) multiplier before reaching the output (measured rel-fro
err 1.6e-2 vs the 2e-2 budget).

Layouts (host-prepped):
  - xT     [768, 2048] f16    x transposed, contraction on partitions
  - WbT    [768, 1024] f16    stationary lhsT tiles for layer 1
  - WHF    [2, 1024, 512] f16 1.7159*W{1,2}.T, moving rhs for ff heads
  - WHT8   [2, 8, 128, 512] f8e4  Q(1024*1.7159*W{a,t}.T) k-subtiled for
                                  DoubleRow (pairs of 128-unit subtiles)
  - BBP    [128, 8] f32       0.666*bb per unit-tile column (ACT bias)
  - TSP    [128, 16] f32      ts, column mi = batch subtile mi
Layer 1 produces hT [units, batch] tiles directly: fp16 for the ff heads
plus an fp8 copy in a [128, 8, chunk] k-subtile-major tile so DoubleRow
lhsT slices [:, 2g:2g+2, m*128:(m+1)*128] need no data movement. The
1/1024 descale of the t-path rides the sigmoid's ACT input scale.

DMA descriptor issues (~0.6us each on the issuing queue) are spread over
the sync/scalar/vector/gpsimd queues so the startup x+weight loads and the
output stores never serialize behind one engine.

The harness' biases are all zero; kernel() builds a no-bias program for
that case (saves 4 DVE adds per batch subtile) and a general program
otherwise.
"""

import os
import sys

import numpy as np

if "/opt/trn_rl_repo" not in sys.path:
    sys.path.insert(0, "/opt/trn_rl_repo")

B, IN, HID, UNITS = 16384, 256, 512, 1024
CAT = IN + HID  # 768
N_CORES = 8
BS = B // N_CORES  # 2048 per core
P = 128
NK1 = CAT // P    # 6 contraction tiles, layer 1
NU = UNITS // P   # 8 unit tiles
SW8 = 1024.0      # fp8 t-head weight scale (power of two)

_cache = {}


def build_nc(bs=BS, chunk=512, has_bias=False):
    """Build the single-core Bass program (same program runs SPMD on 8 cores)."""
    from concourse import bacc, tile, mybir

    AF = mybir.ActivationFunctionType
    ALU = mybir.AluOpType
    PM = mybir.MatmulPerfMode
    F32 = mybir.dt.float32
    F16 = mybir.dt.float16
    F8 = mybir.dt.float8e4

    nchunk = bs // chunk
    nm = chunk // P  # batch subtiles per chunk

    nc = bacc.Bacc("TRN2", target_bir_lowering=False, debug=False,
                   num_devices=N_CORES)

    xt_d = nc.dram_tensor("xt", [CAT, bs], F16, kind="ExternalInput").ap()
    wbt_d = nc.dram_tensor("wbt", [CAT, UNITS], F16, kind="ExternalInput").ap()
    whf_d = nc.dram_tensor("whf", [2, UNITS, HID], F16, kind="ExternalInput").ap()
    wht8_d = nc.dram_tensor("wht8", [2, NU, P, HID], F8, kind="ExternalInput").ap()
    bbp_d = nc.dram_tensor("bbp", [P, NU], F32, kind="ExternalInput").ap()
    tsp_d = nc.dram_tensor("tsp", [P, bs // P], F32, kind="ExternalInput").ap()
    if has_bias:
        bhf_d = nc.dram_tensor("bhf", [2, P, HID], F32, kind="ExternalInput").ap()
        bht_d = nc.dram_tensor("bht", [2, P, HID], F32, kind="ExternalInput").ap()
    out_d = nc.dram_tensor("out", [bs, HID], F16, kind="ExternalOutput").ap()

    with tile.TileContext(nc) as tc:
        with (
            tc.tile_pool(name="const", bufs=1) as const,
            tc.tile_pool(name="xp", bufs=4) as xp,
            tc.tile_pool(name="hp", bufs=4) as hp,
            tc.tile_pool(name="hp8", bufs=4) as hp8,
            tc.tile_pool(name="tp", bufs=2) as tp,
            tc.tile_pool(name="op", bufs=3) as op,
            tc.tile_pool(name="psp", bufs=8, space="PSUM") as psp,
        ):
            # --- first-chunk inputs. Each hardware DMA queue moves these
            # 1KB-row transfers at only ~40-75 GB/s, so arrival order must
            # match consumption order and wb is split across two queues:
            # x tiles on sync; wb halves with all h=0 halves (needed first)
            # before any h=1 half, even c on scalar / odd c on gpsimd.
            HALF = UNITS // 2
            xts0 = []
            for c in range(NK1):
                t = xp.tile([P, chunk], F16, tag=f"x{c}")
                nc.sync.dma_start(t[:], xt_d[c * P:(c + 1) * P, 0:chunk])
                xts0.append(t)
            wb_sb = [[None, None] for _ in range(NK1)]
            bb_sb = const.tile([P, NU], F32, tag="bb")
            ts_sb = const.tile([P, bs // P], F32, tag="ts")
            for h in range(2):
                for c in range(NK1):
                    eng = nc.scalar if c % 2 == 0 else nc.gpsimd
                    t = const.tile([P, HALF], F16, tag=f"wbh{c}_{h}")
                    rows = wbt_d[c * P:(c + 1) * P, h * HALF:(h + 1) * HALF]
                    if h == 0 and c <= 1:
                        # quarter the two first-needed halves so the j=0
                        # matmul's stationary lands ~4us sooner
                        for j in range(4):
                            eng.dma_start(t[:, j * P:(j + 1) * P],
                                          rows[:, j * P:(j + 1) * P])
                    else:
                        eng.dma_start(t[:], rows)
                    wb_sb[c][h] = t
                if h == 0:
                    # between the wb rounds: bb gates the first layer-1
                    # activation (~18us), ts only the first t-path (~55us)
                    nc.scalar.dma_start(bb_sb[:], bbp_d[:])
                    nc.gpsimd.dma_start(ts_sb[:], tsp_d[:])

            # all remaining x chunks next on sync: layer-1 for every chunk
            # runs before any layer-2, so head weights are needed ~50us in
            def load_x(bc):
                xts = []
                for c in range(NK1):
                    t = xp.tile([P, chunk], F16, tag=f"x{c}")
                    nc.sync.dma_start(
                        t[:], xt_d[c * P:(c + 1) * P, bc * chunk:(bc + 1) * chunk])
                    xts.append(t)
                return xts

            xts_all = [xts0] + [load_x(bc) for bc in range(1, nchunk)]

            # head weights + biases on the otherwise-idle gpsimd queue
            wf_sb = [None, None]   # ff heads, fp16 [128, 512] x NU
            wt8_sb = [None, None]  # t heads, fp8 [128, NU, 512] k-subtiled
            for k in range(2):
                row = []
                for u in range(NU):
                    t = const.tile([P, HID], F16, tag=f"wf{k}_{u}")
                    nc.gpsimd.dma_start(t[:], whf_d[k, u * P:(u + 1) * P, :])
                    row.append(t)
                wf_sb[k] = row
            for k in range(2):
                t = const.tile([P, NU, HID], F8, tag=f"wt8{k}")
                for u in range(NU):
                    nc.gpsimd.dma_start(t[:, u, :], wht8_d[k, u])
                wt8_sb[k] = t
            if has_bias:
                bhf_sb = [None, None]
                bht_sb = [None, None]
                for k in range(2):
                    t = const.tile([P, HID], F32, tag=f"bhf{k}")
                    nc.gpsimd.dma_start(t[:], bhf_d[k])
                    bhf_sb[k] = t
                for k in range(2):
                    t = const.tile([P, HID], F32, tag=f"bht{k}")
                    nc.gpsimd.dma_start(t[:], bht_d[k])
                    bht_sb[k] = t

            def layer1(xts):
                """hT = tanh(0.666*(WbT.T @ xT) + 0.666*bb): fp16 tiles per
                u for the ff heads + one fp8 [128, NU, chunk] k-subtile-
                major tile for the DoubleRow t-path.

                c-outer accumulation in two u-half-groups: the first matmul
                only needs xts[0] + wb half 0, so the PE starts as soon as
                the first ~0.26 MB of DMA lands.
                """
                hts = []
                h8 = hp8.tile([P, NU, chunk], F8, tag="h8")
                for h in range(2):
                    pss = [psp.tile([P, chunk], F32, tag="ps", name=f"psl1_{j}")
                           for j in range(NU // 2)]
                    for c in range(NK1):
                        for j in range(NU // 2):
                            nc.tensor.matmul(
                                pss[j][:],
                                wb_sb[c][h][:, j * P:(j + 1) * P],
                                xts[c][:],
                                start=(c == 0), stop=(c == NK1 - 1))
                    for j in range(NU // 2):
                        u = h * (NU // 2) + j
                        ht = hp.tile([P, chunk], F16, tag=f"h{u}")
                        nc.scalar.activation(ht[:], pss[j][:], AF.Tanh,
                                             bias=bb_sb[:, u:u + 1], scale=0.666)
                        # fp8 copy for the t-path on the (layer-1-idle) DVE;
                        # also halves ACT load so psum banks recycle faster
                        nc.vector.tensor_copy(h8[:, u, :], ht[:])
                        hts.append(ht)
                return hts, h8

            def t_path(hts, h8, m, mi):
                """t = sigmoid((pa*ts + pb)/SW8) via fp8 DoubleRow matmuls."""
                msl = slice(m * P, (m + 1) * P)
                ps = [None, None]
                for k in range(2):
                    p = psp.tile([P, HID], F32, tag="ps")
                    for g in range(NU // 2):
                        nc.tensor.matmul(
                            p[:],
                            h8[:, 2 * g:2 * g + 2, msl],
                            wt8_sb[k][:, 2 * g:2 * g + 2, :],
                            start=(g == 0), stop=(g == NU // 2 - 1),
                            perf_mode=PM.DoubleRow)
                    ps[k] = p
                pa, pb = ps
                if has_bias:
                    ua = tp.tile([P, HID], F32, tag="ua")
                    nc.vector.tensor_add(ua[:], pa[:], bht_sb[0][:])
                    ub = tp.tile([P, HID], F32, tag="ub")
                    nc.vector.tensor_add(ub[:], pb[:], bht_sb[1][:])
                    w = tp.tile([P, HID], F32, tag="w")
                    nc.vector.scalar_tensor_tensor(
                        w[:], ua[:], ts_sb[:, mi:mi + 1], ub[:],
                        op0=ALU.mult, op1=ALU.add)
                else:
                    # DVE instructions may read at most one PSUM operand, so
                    # split (pa*ts + pb) into two single-PSUM-source ops
                    w1 = tp.tile([P, HID], F32, tag="w1")
                    nc.vector.tensor_scalar_mul(w1[:], pa[:],
                                                ts_sb[:, mi:mi + 1])
                    w = tp.tile([P, HID], F32, tag="w")
                    nc.vector.tensor_add(w[:], w1[:], pb[:])
                tt = tp.tile([P, HID], F16, tag="tt")
                nc.scalar.activation(tt[:], w[:], AF.Sigmoid, scale=1.0 / SW8)
                return tt

            def ff_psum(hts, m, k):
                ps = psp.tile([P, HID], F32, tag="ps")
                for u in range(NU):
                    nc.tensor.matmul(
                        ps[:],
                        hts[u][:, m * P:(m + 1) * P],
                        wf_sb[k][u][:],
                        start=(u == 0), stop=(u == NU - 1))
                return ps

            def ff_act(ps, k, cs):
                if has_bias:
                    u = tp.tile([P, HID], F32, tag=f"u{k}")
                    nc.vector.tensor_add(u[:, cs], ps[:, cs], bhf_sb[k][:, cs])
                    src = u
                else:
                    src = ps
                f = tp.tile([P, HID], F16, tag=f"f{k}")
                nc.scalar.activation(f[:, cs], src[:, cs], AF.Tanh)
                return f

            def layer2(hts, h8, bc):
                for m in range(nm):
                    mi = bc * nm + m
                    last = (bc == nchunk - 1) and (m == nm - 1)

                    # t-path first so the sigmoid chain overlaps the ff mms
                    tt = t_path(hts, h8, m, mi)

                    p1 = ff_psum(hts, m, 0)
                    f1 = ff_act(p1, 0, slice(0, HID))

                    p2 = ff_psum(hts, m, 1)
                    o = op.tile([P, HID], F16, tag="o")
                    d = tp.tile([P, HID], F16, tag="d")
                    # split the trailing chain into column halves on the
                    # very last tile (ACT/DVE/DMA pipeline out the tail),
                    # with the two out stores issued on different queues
                    ncs = 2 if last else 1
                    for q in range(ncs):
                        cs = slice(q * HID // ncs, (q + 1) * HID // ncs)
                        f2 = ff_act(p2, 1, cs)
                        # o = f1 + tt*(f2 - f1)
                        nc.vector.tensor_sub(d[:, cs], f2[:, cs], f1[:, cs])
                        nc.vector.tensor_mul(d[:, cs], d[:, cs], tt[:, cs])
                        nc.vector.tensor_add(o[:, cs], d[:, cs], f1[:, cs])
                        eng = nc.scalar if q % 2 else nc.sync
                        eng.dma_start(out_d[mi * P:(mi + 1) * P, cs], o[:, cs])

            # --- all layer-1 chunks first, then all layer-2 --------------
            l1 = [layer1(x) for x in xts_all]
            for bc in range(nchunk):
                layer2(l1[bc][0], l1[bc][1], bc)

    nc.compile()
    return nc


def _prep_inputs(input, hx, ts, Wb, bb, W1, b1, W2, b2, Wa, ba, Wt, bt,
                 has_bias, bs=BS, n_cores=N_CORES):
    import ml_dtypes

    f = np.float32
    h = np.float16

    def q8(a):
        return np.asarray(np.clip(a, -240.0, 240.0), ml_dtypes.float8_e4m3)

    x = np.concatenate([np.asarray(input, f), np.asarray(hx, f)], axis=1)
    WbT = np.ascontiguousarray(np.asarray(Wb, f).T.astype(h))   # [768, 1024]
    WHF = np.stack([np.ascontiguousarray((1.7159 * np.asarray(W, f)).T.astype(h))
                    for W in (W1, W2)])                         # [2, 1024, 512]
    WHT8 = np.stack([
        q8((SW8 * 1.7159) * np.asarray(W, f).T).reshape(NU, P, HID)
        for W in (Wa, Wt)])                                     # [2, 8, 128, 512]
    BBP = np.ascontiguousarray(
        (0.666 * np.asarray(bb, f)).reshape(NU, P).T)           # [128, 8]
    ts = np.asarray(ts, f).reshape(-1)
    xh = x.astype(h)

    in_maps = []
    for c in range(n_cores):
        lo, hi = c * bs, (c + 1) * bs
        m = {
            "xt": np.ascontiguousarray(xh[lo:hi].T),            # [768, bs] fp16
            "wbt": WbT,
            "whf": WHF,
            "wht8": WHT8,
            "bbp": BBP,
            "tsp": np.ascontiguousarray(ts[lo:hi].reshape(bs // P, P).T),
        }
        if has_bias:
            m["bhf"] = np.stack([
                np.ascontiguousarray(np.broadcast_to(np.asarray(b, f), (P, HID)))
                for b in (b1, b2)])
            m["bht"] = np.stack([
                np.ascontiguousarray(np.broadcast_to(SW8 * np.asarray(b, f),
                                                     (P, HID)))
                for b in (ba, bt)])
        in_maps.append(m)
    return in_maps


def kernel(input, hx, ts, Wb, bb, W1, b1, W2, b2, Wa, ba, Wt, bt):
    from concourse.bass_utils import run_bass_kernel_spmd

    has_bias = bool(np.any(bb) or np.any(b1) or np.any(b2)
                    or np.any(ba) or np.any(bt))
    key = f"nc{int(has_bias)}"
    if key not in _cache:
        _cache[key] = build_nc(has_bias=has_bias)
    nc = _cache[key]

    in_maps = _prep_inputs(input, hx, ts, Wb, bb, W1, b1, W2, b2, Wa, ba, Wt,
                           bt, has_bias)
    trace = bool(int(os.environ.get("KERNEL_PROFILE", "0")))
    res = run_bass_kernel_spmd(nc, in_maps, list(range(N_CORES)), trace=trace)
    _cache["last_exec_time_ns"] = res.exec_time_ns
    _cache["last_results"] = res

    out = np.concatenate([np.asarray(res.results[c]["out"])
                          for c in range(N_CORES)], axis=0)
    return out.astype(np.float32)
